# revision 7
# baseline (speedup 1.0000x reference)
"""Trainium2 Bass kernel for nn_LlamaAttention (GQA prefill attention, paged-cache
semantics, RoPE) on 8 NeuronCores.

Sharding: token-parallel, no collectives. Core c handles sequence c//2, query
half c%2 (512 query tokens, all 32 heads, 1024 keys of its sequence). Each core
runs an identical SPMD program; all position/causality information is passed as
per-core data (local token ordering, masks, cos/sin tables).

Device-side dataflow (per core), all matmuls in float32r (TF32-like, 1 cyc/row):
  A : PE-transpose hidden chunks -> hT [hid, tok]; K/V projections (transposed
      layouts KT/VT [hd, tok]); RoPE on K via partition-rotate DMA + DVE;
      V re-transposed to natural [tok, hd] via PE.
  A2: Q projection for the 512 query tokens -> RoPE -> spill QT to DRAM.
  B : per q-head: S_T[k,q] = KT_tile.T @ qt; exp (ACT); causal mask multiply
      (DVE); denominators via ones-matmul (partition reduction); PV matmul
      accumulates attn_T[hd,q]; normalize with reciprocal broadcast by K=1
      matmul.
  C : o_proj: out[tok, oc] accumulated over 32 head-blocks in PSUM,
      lhsT = attn_T (already transposed), rhs = Wo row-block (streamed).
"""
import sys

sys.path.insert(0, "/opt/trn_rl_repo")

import numpy as np

B, S, D = 4, 1024, 4096
NH, NKV, HD = 32, 8, 128
G = NH // NKV
T = B * S
HALF = HD // 2
ROPE_BASE = 10000.0
N_CORES = 8

_prog = None


def _build_program():
    import concourse.bass as bass
    import concourse.tile as tile
    from concourse import bacc, mybir
    from concourse.masks import make_identity

    F32, F32R = mybir.dt.float32, mybir.dt.float32r
    AFT = mybir.ActivationFunctionType

    nc = bacc.Bacc()
    hs_d = nc.declare_dram_parameter("hs", [1024, D], F32, isOutput=False)
    wq_d = nc.declare_dram_parameter("wq", [NH, 128, 32, 128], F32R, isOutput=False)
    wk_d = nc.declare_dram_parameter("wk", [NKV, 128, 32, 128], F32R, isOutput=False)
    wv_d = nc.declare_dram_parameter("wv", [NKV, 128, 32, 128], F32R, isOutput=False)
    wo_d = nc.declare_dram_parameter("wo", [D, D], F32R, isOutput=False)
    cos_d = nc.declare_dram_parameter("cosT", [128, 1024], F32, isOutput=False)
    sin_d = nc.declare_dram_parameter("sinT", [128, 1024], F32, isOutput=False)
    msk_d = nc.declare_dram_parameter("maskT", [128, 8, 512], F32, isOutput=False)
    out_d = nc.declare_dram_parameter("out", [512, D], F32, isOutput=True)
    qsp_d = nc.dram_tensor("qspill", [NH, 128, 512], F32R)

    with tile.TileContext(nc) as tc:
        with tc.tile_pool(name="const", bufs=1) as const, \
             tc.tile_pool(name="persist", bufs=1) as persist:
            ident = const.tile([128, 128], F32)
            make_identity(nc, ident[:])
            ones_f32 = const.tile([128, 128], F32)
            nc.gpsimd.memset(ones_f32[:], 1.0)
            ones_col = const.tile([128, 1], F32R)
            nc.vector.tensor_copy(ones_col[:], ones_f32[:, 0:1])
            ones_row = const.tile([1, 128], F32R)
            nc.vector.tensor_copy(ones_row[:], ones_f32[0:1, :])
            cos_t = const.tile([128, 1024], F32)
            nc.sync.dma_start(cos_t[:], cos_d[:])
            sin_t = const.tile([128, 1024], F32)
            nc.sync.dma_start(sin_t[:], sin_d[:])

            KT = persist.tile([128, NKV, 1024], F32R)      # [hd, v, ktok]
            Vn = persist.tile([128, NKV, 8, 128], F32R)    # [tokp, v, ktile, hd]

            def rope(dst, src, shift, t1, col0, n):
                # dst = src*cos + rotate64(src)*sin' (sin sign-folded on host)
                nc.sync.dma_start(shift[0:HALF, :], src[HALF:128, :])
                nc.sync.dma_start(shift[HALF:128, :], src[0:HALF, :])
                nc.vector.tensor_mul(t1[:], src[:], cos_t[:, col0:col0 + n])
                nc.vector.tensor_mul(shift[:], shift[:], sin_t[:, col0:col0 + n])
                nc.vector.tensor_add(dst, t1[:], shift[:])

            # ---------------- Phase A / A2 ----------------
            with tc.tile_pool(name="ph_a", bufs=1) as pa, \
                 tc.tile_pool(name="hload", bufs=2) as hload, \
                 tc.tile_pool(name="wtile", bufs=2) as wpool, \
                 tc.tile_pool(name="atmp", bufs=2) as atmp, \
                 tc.tile_pool(name="ps_t", bufs=4, space="PSUM") as ps_t, \
                 tc.tile_pool(name="ps_p", bufs=2, space="PSUM") as ps_p:
                hT = pa.tile([128, 32, 512], F32R)  # current chunk hidden^T
                for chunk in (1, 0):  # chunk 0 last: hT holds q-chunk for A2
                    for tt in range(4):
                        for colh in range(2):
                            hn = hload.tile([128, 2048], F32, tag="hn")
                            nc.sync.dma_start(
                                hn[:],
                                hs_d[chunk * 512 + tt * 128:chunk * 512 + (tt + 1) * 128,
                                     colh * 2048:(colh + 1) * 2048])
                            for k2 in range(16):
                                kt = colh * 16 + k2
                                pt = ps_t.tile([128, 128], F32, tag="pt")
                                nc.tensor.transpose(pt[:], hn[:, k2 * 128:(k2 + 1) * 128], ident[:])
                                nc.vector.tensor_copy(hT[:, kt, tt * 128:(tt + 1) * 128], pt[:])
                    for v in range(NKV):
                        wk_t = wpool.tile([128, 32, 128], F32R, tag="w")
                        nc.sync.dma_start(wk_t[:], wk_d[v])
                        psK = ps_p.tile([128, 512], F32, tag="pp")
                        for kt in range(32):
                            nc.tensor.matmul(psK[:], wk_t[:, kt], hT[:, kt],
                                             start=kt == 0, stop=kt == 31)
                        kraw = atmp.tile([128, 512], F32, tag="raw")
                        nc.scalar.copy(kraw[:], psK[:])
                        shift = atmp.tile([128, 512], F32, tag="shift")
                        t1 = atmp.tile([128, 512], F32, tag="t1")
                        rope(KT[:, v, chunk * 512:(chunk + 1) * 512], kraw, shift, t1,
                             chunk * 512, 512)

                        wv_t = wpool.tile([128, 32, 128], F32R, tag="w")
                        nc.sync.dma_start(wv_t[:], wv_d[v])
                        psV = ps_p.tile([128, 512], F32, tag="pp")
                        for kt in range(32):
                            nc.tensor.matmul(psV[:], wv_t[:, kt], hT[:, kt],
                                             start=kt == 0, stop=kt == 31)
                        vraw = atmp.tile([128, 512], F32, tag="raw")
                        nc.scalar.copy(vraw[:], psV[:])
                        for st in range(4):
                            pt = ps_t.tile([128, 128], F32, tag="pt")
                            nc.tensor.transpose(pt[:], vraw[:, st * 128:(st + 1) * 128], ident[:])
                            nc.vector.tensor_copy(Vn[:, v, chunk * 4 + st, :], pt[:])
                # A2: Q projection for q-chunk (chunk 0, currently in hT)
                for h in range(NH):
                    wq_t = wpool.tile([128, 32, 128], F32R, tag="w")
                    nc.sync.dma_start(wq_t[:], wq_d[h])
                    psQ = ps_p.tile([128, 512], F32, tag="pp")
                    for kt in range(32):
                        nc.tensor.matmul(psQ[:], wq_t[:, kt], hT[:, kt],
                                         start=kt == 0, stop=kt == 31)
                    qraw = atmp.tile([128, 512], F32, tag="raw")
                    nc.scalar.copy(qraw[:], psQ[:])
                    shift = atmp.tile([128, 512], F32, tag="shift")
                    qf = atmp.tile([128, 512], F32R, tag="qf")
                    t1q = atmp.tile([128, 512], F32, tag="t1")
                    rope(qf[:], qraw, shift, t1q, 0, 512)
                    nc.sync.dma_start(qsp_d[h], qf[:])

            # ---------------- Phase B / C ----------------
            with tc.tile_pool(name="attnp", bufs=1) as apool:
                attnT = apool.tile([128, NH, 512], F32R)
                with tc.tile_pool(name="bmask", bufs=1) as bm, \
                     tc.tile_pool(name="qload", bufs=3) as qload, \
                     tc.tile_pool(name="es", bufs=4) as espool, \
                     tc.tile_pool(name="btmp", bufs=2) as btmp, \
                     tc.tile_pool(name="ps_s", bufs=3, space="PSUM") as ps_s, \
                     tc.tile_pool(name="ps_a", bufs=2, space="PSUM") as ps_a, \
                     tc.tile_pool(name="ps_d", bufs=1, space="PSUM") as ps_d, \
                     tc.tile_pool(name="ps_b", bufs=1, space="PSUM") as ps_b:
                    maskT = bm.tile([128, 8, 512], F32)
                    nc.sync.dma_start(maskT[:], msk_d[:])
                    for h in range(NH):
                        v = h // G
                        qt = qload.tile([128, 512], F32R, tag="qt")
                        nc.sync.dma_start(qt[:], qsp_d[h])
                        psA = ps_a.tile([128, 512], F32, tag="pa")
                        psD = ps_d.tile([1, 512], F32, tag="pd")
                        for kt in range(8):
                            psS = ps_s.tile([128, 512], F32, tag="psS")
                            nc.tensor.matmul(psS[:], KT[:, v, kt * 128:(kt + 1) * 128],
                                             qt[:], start=True, stop=True)
                            ex = espool.tile([128, 512], F32, tag="ex")
                            nc.scalar.activation(ex[:], psS[:], AFT.Exp)
                            exr = espool.tile([128, 512], F32R, tag="exr")
                            nc.vector.tensor_mul(exr[:], ex[:], maskT[:, kt])
                            nc.tensor.matmul(psD[:], ones_col[:], exr[:],
                                             start=kt == 0, stop=kt == 7)
                            nc.tensor.matmul(psA[:], Vn[:, v, kt], exr[:],
                                             start=kt == 0, stop=kt == 7)
                        den = btmp.tile([1, 512], F32R, tag="den")
                        with nc.allow_low_precision(reason="f32r bits are fp32"):
                            nc.vector.reciprocal(den[:], psD[:])
                        psB = ps_b.tile([128, 512], F32, tag="pb")
                        nc.tensor.matmul(psB[:], ones_row[:], den[:], start=True, stop=True)
                        rb = btmp.tile([128, 512], F32, tag="rb")
                        nc.scalar.copy(rb[:], psB[:])
                        nc.vector.tensor_mul(attnT[:, h], psA[:], rb[:])

                with tc.tile_pool(name="wop", bufs=4) as wop, \
                     tc.tile_pool(name="osb", bufs=2) as osb, \
                     tc.tile_pool(name="ps_o", bufs=2, space="PSUM") as ps_o:
                    out_r = out_d.rearrange("(tt p) o -> p tt o", p=128)
                    for oc in range(8):
                        pso = [ps_o.tile([128, 512], F32, tag=f"o{tt}", name=f"pso{tt}")
                               for tt in range(4)]
                        for h in range(NH):
                            wot = wop.tile([128, 512], F32R, tag="wo")
                            nc.sync.dma_start(
                                wot[:], wo_d[h * 128:(h + 1) * 128, oc * 512:(oc + 1) * 512])
                            for tt in range(4):
                                nc.tensor.matmul(pso[tt][:],
                                                 attnT[:, h, tt * 128:(tt + 1) * 128],
                                                 wot[:], start=h == 0, stop=h == NH - 1)
                        ot = osb.tile([128, 4, 512], F32, tag="ot")
                        for tt in range(4):
                            nc.scalar.copy(ot[:, tt], pso[tt][:])
                        nc.sync.dma_start(out_r[:, :, oc * 512:(oc + 1) * 512], ot[:])

    nc.finalize()
    return nc


def _get_program():
    global _prog
    if _prog is None:
        _prog = _build_program()
    return _prog


def _host_prep(hidden_states, Wq, Wk, Wv, Wo, position_ids):
    """Returns (shared_inputs, per_core_inputs, q_rows_per_core)."""
    hs = np.ascontiguousarray(hidden_states, dtype=np.float32)
    Wq_s = (np.asarray(Wq, np.float64) / np.sqrt(HD)).astype(np.float32)
    # [h, p, kt, c] relayout so each [128,32,128] head-tile DMA has 16KB lines
    wq_r = np.ascontiguousarray(Wq_s.reshape(32, 128, NH, 128).transpose(2, 1, 0, 3))
    wk_r = np.ascontiguousarray(
        np.asarray(Wk, np.float32).reshape(32, 128, NKV, 128).transpose(2, 1, 0, 3))
    wv_r = np.ascontiguousarray(
        np.asarray(Wv, np.float32).reshape(32, 128, NKV, 128).transpose(2, 1, 0, 3))
    wo = np.ascontiguousarray(Wo, dtype=np.float32)
    pos = np.asarray(position_ids, np.int64)

    inv_freq = 1.0 / (ROPE_BASE ** (np.arange(HALF, dtype=np.float64) / HALF))
    sgn = np.where(np.arange(HD) < HALF, -1.0, 1.0)

    shared = dict(wq=wq_r, wk=wk_r, wv=wv_r, wo=wo)
    per_core = []
    q_rows_all = []
    for c in range(N_CORES):
        seq, qhalf = c // 2, c % 2
        rows_seq = np.arange(seq * S, (seq + 1) * S)
        q_rows = rows_seq[qhalf * 512:(qhalf + 1) * 512]
        o_rows = rows_seq[(1 - qhalf) * 512:(2 - qhalf) * 512]
        k_rows = np.concatenate([q_rows, o_rows])  # local order: q-chunk first

        pos_k = pos[k_rows]
        freqs = pos_k[:, None].astype(np.float64) * inv_freq[None, :]
        emb = np.concatenate([freqs, freqs], axis=1)          # [1024, 128]
        cosT = np.ascontiguousarray(np.cos(emb).T).astype(np.float32)
        sinT = np.ascontiguousarray((np.sin(emb) * sgn[None, :]).T).astype(np.float32)

        q_rowidx = q_rows - seq * S
        maskT = (pos_k[:, None] <= q_rowidx[None, :]).astype(np.float32)  # [1024,512]
        maskT = np.ascontiguousarray(maskT.reshape(8, 128, 512).transpose(1, 0, 2))

        per_core.append(dict(hs=np.ascontiguousarray(hs[k_rows]),
                             cosT=cosT, sinT=sinT, maskT=maskT, **shared))
        q_rows_all.append(q_rows)
    return per_core, q_rows_all


def kernel(hidden_states, Wq, Wk, Wv, Wo, k_cache, v_cache,
           position_ids, block_offsets, _trace=False):
    from concourse.bass_utils import run_bass_kernel_spmd

    nc = _get_program()
    per_core, q_rows_all = _host_prep(hidden_states, Wq, Wk, Wv, Wo, position_ids)
    res = run_bass_kernel_spmd(nc, per_core, list(range(N_CORES)), trace=_trace)
    out = np.zeros((T, D), np.float32)
    for c in range(N_CORES):
        out[q_rows_all[c]] = res.results[c]["out"]
    if _trace:
        kernel._last_results = res
    return out


if __name__ == "__main__":
    rng = np.random.default_rng(0)
    ins = dict(
        hidden_states=rng.standard_normal((T, D), dtype=np.float32) * 0.02,
        Wq=rng.standard_normal((D, NH * HD), dtype=np.float32) / np.sqrt(D),
        Wk=rng.standard_normal((D, NKV * HD), dtype=np.float32) / np.sqrt(D),
        Wv=rng.standard_normal((D, NKV * HD), dtype=np.float32) / np.sqrt(D),
        Wo=rng.standard_normal((NH * HD, D), dtype=np.float32) / np.sqrt(NH * HD),
        k_cache=np.zeros((80, 64, 8, 128), np.float32),
        v_cache=np.zeros((80, 64, 8, 128), np.float32),
        position_ids=np.tile(np.arange(S, dtype=np.int32), B),
        block_offsets=np.arange(B * 16, dtype=np.int32).reshape(B, 16),
    )
    out = kernel(**ins)
    print("ran ok", out.shape, out.dtype, float(np.abs(out).mean()))
